# revision 1
# baseline (speedup 1.0000x reference)
"""Trainium2 Bass kernel for nn_CVAE decoder LSTM.

Data-parallel over 8 NeuronCores: batch 8192 -> 1024 per core.

Math (per core, B=1024, T=2048, H=128):
  Reference step:  gates = x_t @ Wx.T + static_proj + h @ W_hh.T
                   i,f,g,o = split(gates); c' = sig(f)*c + sig(i)*tanh(g)
                   h' = sig(o)*tanh(c'); x' = h' @ W_out.T + b_out; ys[t] = x'
  Since x_t = h_t @ W_out.T + b_out for t>=1, fold:
    W_comb = W_hh.T + W_out.T @ Wx.T            [128, 512]
    P0     = static_proj + b_out @ Wx.T         [B, 512]  (constant per step)
    gates_t = h_t @ W_comb + P0                 (t >= 1)
    gates_0 = static_proj + start @ Wx.T = P_first (h_0 = 0, no matmul)

On-chip layout: unit-major, gate order permuted to [i, f, o, g] so sigmoid
covers a contiguous [3*BG] region and tanh the trailing [BG].
Batch is split into NG=4 independent groups of BG=256 whose serial cell
chains stagger across engines (PE -> ACT -> DVE/GP -> ACT -> GP -> PE).
Gates per group: one PSUM tile [128, 4, BG] (2 banks); P0 added via
identity-matmul accumulation. h,c state: per-group tiles [128, BG].
Output x' computed batch-major via matmul(lhsT=h_tile, rhs=W_out.T),
collected in SBUF chunks of U=64 steps, DMA'd to DRAM as [b, t, 2].
"""

import numpy as np

import concourse.bass as bass
import concourse.bacc as bacc
import concourse.tile as tile
from concourse import mybir
from concourse.bass_utils import run_bass_kernel_spmd

f32 = mybir.dt.float32
f32r = mybir.dt.float32r
AF = mybir.ActivationFunctionType

HIDDEN = 128
INPUT_SIZE = 2
T = 2048
B_FULL = 8192
NCORES = 8
BC = B_FULL // NCORES      # 1024 batch rows per core
NG = 4                     # independent batch groups per core
BG = BC // NG              # 256 rows per group
NBT = BG // 128            # 2 batch-tiles of 128 per group
import os
U = int(os.environ.get("K_U", "128"))  # steps per loop chunk
NCHUNK = T // U            # chunks (chunk 0 peeled)
NO_X = os.environ.get("K_NO_X", "") == "1"
STAGGER = os.environ.get("K_STAGGER", "1") == "1"



def _build_nc(t_total=T):
    nc = bacc.Bacc("TRN2", target_bir_lowering=False)

    p0_d = nc.dram_tensor("p0", [4, HIDDEN, BC], f32r, kind="ExternalInput")
    pf_d = nc.dram_tensor("pf", [4, HIDDEN, BC], f32, kind="ExternalInput")
    wcomb_d = nc.dram_tensor("wcomb", [HIDDEN, 4 * HIDDEN], f32r, kind="ExternalInput")
    woutT_d = nc.dram_tensor("woutT", [HIDDEN, INPUT_SIZE], f32r, kind="ExternalInput")
    ident_d = nc.dram_tensor("ident", [HIDDEN, HIDDEN], f32r, kind="ExternalInput")
    bout_d = nc.dram_tensor("bout", [1, NBT * INPUT_SIZE], f32, kind="ExternalInput")
    y_d = nc.dram_tensor("y", [BC, t_total, INPUT_SIZE], f32, kind="ExternalOutput")
    # [p, k, t, j] view of y: batch row b = k*128 + p
    y_v = y_d.ap().rearrange("(k p) t j -> p k t j", p=128)

    with tile.TileContext(nc) as tc:
        with (
            tc.tile_pool(name="consts", bufs=1) as consts,
            tc.tile_pool(name="hpool", bufs=2 * NG) as hpool,
            tc.tile_pool(name="cpool", bufs=2 * NG) as cpool,
            tc.tile_pool(name="cell", bufs=2 * NG) as cell,
            tc.tile_pool(name="xbuf", bufs=1) as xbuf,
            tc.tile_pool(name="ps_gate", bufs=3, space="PSUM") as ps_gate,
            tc.tile_pool(name="ps_x", bufs=2, space="PSUM") as ps_x,
        ):
            # ---- constants ----
            wcomb = consts.tile([HIDDEN, 4 * HIDDEN], f32r)
            woutT = consts.tile([HIDDEN, INPUT_SIZE], f32r)
            ident = consts.tile([HIDDEN, HIDDEN], f32r)
            bout = consts.tile([128, NBT * INPUT_SIZE], f32)
            p0 = consts.tile([128, 4, BC], f32r)   # [hid, gate, batch]
            nc.gpsimd.dma_start(out=wcomb, in_=wcomb_d[:, :])
            nc.gpsimd.dma_start(out=woutT, in_=woutT_d[:, :])
            nc.gpsimd.dma_start(out=ident, in_=ident_d[:, :])
            nc.gpsimd.dma_start(
                out=bout, in_=bout_d.ap().to_broadcast((128, NBT * INPUT_SIZE)))
            nc.gpsimd.dma_start(out=p0, in_=p0_d.ap().rearrange("g p b -> p g b"))

            def group_phase1(gi, h_prev, c_prev, first):
                """Gates + activations + c-update for group gi. Returns (s_ifo, c_new)."""
                bs = slice(gi * BG, (gi + 1) * BG)
                if first:
                    gates = cell.tile([128, 4, BG], f32, tag="gates_f")
                    nc.sync.dma_start(
                        out=gates,
                        in_=pf_d.ap().rearrange("g p b -> p g b")[:, :, bs],
                    )
                else:
                    gates = ps_gate.tile([128, 4, BG], f32)
                    for g in range(4):
                        nc.tensor.matmul(
                            gates[:, g, :],
                            wcomb[:, g * 128:(g + 1) * 128],
                            h_prev[:, :],
                            start=True, stop=False,
                        )
                        nc.tensor.matmul(
                            gates[:, g, :], ident, p0[:, g, bs],
                            start=False, stop=True,
                        )

                # tanh(g) first (slot 3), then sigmoids of i,f,o in one op
                g_t = cell.tile([128, BG], f32, tag="g_t")
                nc.scalar.activation(g_t, gates[:, 3, :], AF.Tanh)
                s_ifo = cell.tile([128, 3, BG], f32, tag="s_ifo")
                nc.scalar.activation(s_ifo, gates[:, 0:3, :], AF.Sigmoid)

                # u = sig(i)*tanh(g) on GPSIMD ; t2 = sig(f)*c on DVE
                u = cell.tile([128, BG], f32, tag="u")
                nc.gpsimd.tensor_mul(u, s_ifo[:, 0, :], g_t)
                t2 = cell.tile([128, BG], f32, tag="t2")
                nc.vector.tensor_mul(t2, s_ifo[:, 1, :], c_prev)
                c_new = cpool.tile([128, BG], f32)
                nc.vector.tensor_add(c_new, t2, u)
                return s_ifo, c_new

            def group_phase2(gi, s_in_chunk, s_ifo, c_new, x_sb):
                """tanh(c), h, and x output for group gi. Returns h_new."""
                tau = cell.tile([128, BG], f32, tag="tau")
                nc.scalar.activation(tau, c_new, AF.Tanh)
                h_new = hpool.tile([128, BG], f32r)
                nc.gpsimd.tensor_mul(h_new, s_ifo[:, 2, :], tau)
                if NO_X:
                    return h_new

                # x' = h' @ W_out.T (batch-major), + b_out, into x_sb
                x_ps = ps_x.tile([128, NBT * INPUT_SIZE], f32)
                for k in range(NBT):
                    nc.tensor.matmul(
                        x_ps[:, 2 * k:2 * k + 2],
                        h_new[:, k * 128:(k + 1) * 128],
                        woutT,
                        start=True, stop=True,
                    )
                nc.vector.tensor_add(
                    x_sb[:, NBT * gi:NBT * (gi + 1), s_in_chunk, :],
                    x_ps.rearrange("p (k j) -> p k j", j=INPUT_SIZE),
                    bout.rearrange("p (k j) -> p k j", j=INPUT_SIZE),
                )
                return h_new

            def step(s_in_chunk, hs, cs, x_sb, first=False):
                ph1 = [group_phase1(gi, hs[gi], cs[gi], first) for gi in range(NG)]
                hs_new = [group_phase2(gi, s_in_chunk, sifo, cnew, x_sb)
                          for gi, (sifo, cnew) in enumerate(ph1)]
                return hs_new, [p[1] for p in ph1]

            # ---- peeled chunk 0 ----
            u0 = min(U, t_total)
            n_chunks = t_total // U if t_total >= U else 1
            cs = []
            for gi in range(NG):
                c0 = cpool.tile([128, BG], f32, name="c0")
                nc.vector.memset(c0, 0.0)
                cs.append(c0)
            hs = [hpool.tile([128, BG], f32r, name="h0") for _ in range(NG)]  # unused @ step0
            x_sb = xbuf.tile([128, NG * NBT, u0, INPUT_SIZE], f32)
            for s in range(u0):
                hs, cs = step(s, hs, cs, x_sb, first=(s == 0))
            if not NO_X:
                nc.sync.dma_start(out=y_v[:, :, 0:u0, :], in_=x_sb)

            # ---- chunks 1..n_chunks-1 ----
            if n_chunks > 1:
                with tc.For_i(1, n_chunks, 1, staggered_reset=STAGGER) as ci:
                    x_sb2 = xbuf.tile([128, NG * NBT, U, INPUT_SIZE], f32)
                    hs2, cs2 = hs, cs
                    for s in range(U):
                        hs2, cs2 = step(s, hs2, cs2, x_sb2, first=False)
                    if not NO_X:
                        nc.sync.dma_start(out=y_v[:, :, bass.ts(ci, U), :], in_=x_sb2)
    nc.compile()
    return nc


_NC_CACHE = {}


def _get_nc():
    if "nc" not in _NC_CACHE:
        _NC_CACHE["nc"] = _build_nc()
    return _NC_CACHE["nc"]


def kernel(z, condition, start_point, W_ih, W_hh, b_ih, b_hh, W_out, b_out, seq_len):
    z = np.asarray(z, dtype=np.float32)
    condition = np.asarray(condition, dtype=np.float32)
    start_point = np.asarray(start_point, dtype=np.float32)
    W_ih = np.asarray(W_ih, dtype=np.float32)
    W_hh = np.asarray(W_hh, dtype=np.float32)
    b_ih = np.asarray(b_ih, dtype=np.float32)
    b_hh = np.asarray(b_hh, dtype=np.float32)
    W_out = np.asarray(W_out, dtype=np.float32)
    b_out = np.asarray(b_out, dtype=np.float32)
    assert int(seq_len) == T and z.shape[0] == B_FULL

    B = z.shape[0]
    dt_col = np.full((B, 1), 0.05, dtype=np.float32)
    static_in = np.concatenate([z, condition, dt_col], axis=-1)          # [B, 37]
    static_proj = static_in @ W_ih[:, INPUT_SIZE:].T + b_ih + b_hh       # [B, 512]
    Wx = W_ih[:, :INPUT_SIZE]                                            # [512, 2]
    P0 = static_proj + b_out @ Wx.T                                      # [B, 512]
    Pf = static_proj + start_point @ Wx.T                                # [B, 512]
    W_comb = (W_hh.T + W_out.T @ Wx.T).astype(np.float32)                # [128, 512]

    # unit-major, per-gate: [4, 128, B]; reorder gates [i,f,g,o] -> [i,f,o,g]
    # (kernel applies sigmoid to slots 0:3 and tanh to slot 3)
    GP = [0, 1, 3, 2]
    P0_t = np.ascontiguousarray(P0.T.reshape(4, HIDDEN, B)[GP], dtype=np.float32)
    Pf_t = np.ascontiguousarray(Pf.T.reshape(4, HIDDEN, B)[GP], dtype=np.float32)
    W_comb = np.ascontiguousarray(
        W_comb.reshape(HIDDEN, 4, HIDDEN)[:, GP, :].reshape(HIDDEN, 4 * HIDDEN)
    )
    woutT = np.ascontiguousarray(W_out.T, dtype=np.float32)              # [128, 2]
    ident = np.eye(HIDDEN, dtype=np.float32)
    bout_rep = np.tile(b_out, NBT)[None, :].astype(np.float32)           # [1, 4]

    nc = _get_nc()
    in_maps = []
    for c in range(NCORES):
        bs = slice(c * BC, (c + 1) * BC)
        in_maps.append({
            "p0": np.ascontiguousarray(P0_t[:, :, bs]),
            "pf": np.ascontiguousarray(Pf_t[:, :, bs]),
            "wcomb": W_comb,
            "woutT": woutT,
            "ident": ident,
            "bout": bout_rep,
        })
    global _last_in_maps
    _last_in_maps = in_maps
    res = run_bass_kernel_spmd(nc, in_maps, core_ids=list(range(NCORES)))
    out = np.concatenate([r["y"] for r in res.results], axis=0)
    return out


_last_in_maps = None

